# revision 5
# baseline (speedup 1.0000x reference)
"""Trainium2 Bass kernel for nn_CoconAttention (dense transformer attention block).

Sharding: 8 cores = 4 batches x 2 head-groups (8 heads each). Each core gets
pre-permuted bf16 inputs ([128, kc, ...] SBUF layouts built on host so DMAs are
fat + contiguous), computes its partial output outT [1024, 896] (transposed,
pre-b_proj), and the host sums head-group pairs + transposes.

v3 (per core, H=8 heads, Dh=64, T=896, Tc=128, S=1024):
  all matmul operands bf16 (psum accumulation fp32); exp on ACT; masks on DVE.
  Software-pipelined emission over three token sweeps (0:512, 512:768,
  768:896): attention(p, sweep0) interleaves qk-proj(p+1) MMs into the PE
  stream; attention(p, sweep k>0) interleaves out-proj(sweep k-1) MMs.
  Denominators for a sweep are final at its end (causal window j <= i+128),
  so normalization + out-proj trail each sweep by one.
"""
import os
import sys
from collections import deque

import numpy as np
import ml_dtypes

try:
    import concourse.bass as bass
except ImportError:  # fresh grading dir: fall back to the repo location
    sys.path.insert(0, "/opt/trn_rl_repo")
    import concourse.bass as bass
import concourse.bacc as bacc

import concourse.tile as tile
from concourse import mybir
from concourse.bass_utils import run_bass_kernel_spmd
from contextlib import ExitStack

F32 = mybir.dt.float32
BF16 = mybir.dt.bfloat16
AF = mybir.ActivationFunctionType
ALU = mybir.AluOpType

T, Tc, NX = 896, 128, 1024
TCH = ((0, 512), (512, 896))  # qk-proj tok chunks (psum-width limited)
SWEEPS = ((0, 512, 4), (512, 768, 6), (768, 896, 7))  # (ts, te, last_c)
NPAIR = 4  # head pairs per core


def _bc0(ap, n):
    """Partition-broadcast read AP: [1, ...] -> [n, ...] with partition step 0."""
    return bass.AP(tensor=ap.tensor, offset=ap.offset, ap=[[0, n]] + list(ap.ap[1:]))


def _rect(c, ts, te):
    """Live (unmasked) column range of scores chunk c within tok range [ts,te)."""
    cs = max(max(0, 128 * (c - 1)), ts)
    return None if cs >= te else (cs, te)


def _band_pieces(c, ts, te):
    """Mask applications for chunk c in [ts,te): (s0, e0, mask_col_offset)."""
    if c == 0:
        bs, be, moff, borig = 0, 128, 128, 0  # diag half only
    elif c <= 6:
        bs = 128 * (c - 1)
        be, moff, borig = bs + 256, 0, bs  # causal(128) + diag(128)
    else:
        bs, be, moff, borig = 768, 896, 0, 768  # causal half only
    s0, e0 = max(bs, ts), min(be, te)
    if s0 >= e0:
        return []
    return [(s0, e0, moff + (s0 - borig))]


def build_nc():
    nc = bacc.Bacc("TRN2", target_bir_lowering=False)

    x_h = nc.dram_tensor("xr", [128, 8, T], BF16, kind="ExternalInput")
    ctx_h = nc.dram_tensor("ctxr", [128, 8, Tc], BF16, kind="ExternalInput")
    wq_h = nc.dram_tensor("w_q", [128, 8, 512], BF16, kind="ExternalInput")
    wk_h = nc.dram_tensor("w_k", [128, 8, 512], BF16, kind="ExternalInput")
    wv_h = nc.dram_tensor("w_v", [128, 8, 512], BF16, kind="ExternalInput")
    wkc_h = nc.dram_tensor("w_kc", [128, 8, 512], BF16, kind="ExternalInput")
    wvc_h = nc.dram_tensor("w_vc", [128, 8, 512], BF16, kind="ExternalInput")
    wpj_h = nc.dram_tensor("w_pj", [128, 4, NX], BF16, kind="ExternalInput")
    bqk_h = nc.dram_tensor("b_qk", [128, 8], F32, kind="ExternalInput")
    bkc_h = nc.dram_tensor("b_kc", [128, 4], F32, kind="ExternalInput")
    bv_h = nc.dram_tensor("b_v", [1, 512], F32, kind="ExternalInput")
    bvc_h = nc.dram_tensor("b_vc", [1, 512], F32, kind="ExternalInput")
    mb_h = nc.dram_tensor("mband", [128, 256], BF16, kind="ExternalInput")
    out_h = nc.dram_tensor("outT", [NX, T], F32, kind="ExternalOutput")

    with tile.TileContext(nc) as tc, ExitStack() as top:
        consts = top.enter_context(tc.tile_pool(name="consts", bufs=1))
        qkp = top.enter_context(tc.tile_pool(name="qkp", bufs=1))
        vtp = top.enter_context(tc.tile_pool(name="vtp", bufs=1))
        atp_sb = top.enter_context(tc.tile_pool(name="atp_sb", bufs=1))
        wts = top.enter_context(tc.tile_pool(name="wts", bufs=1))
        misc = top.enter_context(tc.tile_pool(name="misc", bufs=2))
        probsp = top.enter_context(tc.tile_pool(name="probsp", bufs=4))
        outp = top.enter_context(tc.tile_pool(name="outp", bufs=2))
        dramp = top.enter_context(tc.tile_pool(name="dramp", bufs=2, space="DRAM"))
        psg = top.enter_context(tc.tile_pool(name="psg", bufs=2, space="PSUM"))
        scp = top.enter_context(tc.tile_pool(name="scp", bufs=2, space="PSUM"))
        atps = top.enter_context(tc.tile_pool(name="atps", bufs=1, space="PSUM"))

        # ---- tiny consts + ACT exp table warmup + PE HAM warmup ----
        ebias = consts.tile([128, 2], F32, name="ebias")  # [0]=0, [1]=ctx bias -2
        nc.vector.memset(ebias[:, 0:1], 0.0)
        nc.vector.memset(ebias[:, 1:2], -2.0)
        warm = consts.tile([128, 2], F32, name="warm")
        nc.scalar.activation(out=warm, in_=ebias, func=AF.Exp, scale=1.0)
        ebias_b = consts.tile([128, 2], BF16, name="ebias_b")
        nc.vector.tensor_copy(out=ebias_b, in_=ebias)
        wps = psg.tile([128, 512], F32, tag="gen", name="warmps")
        for _ in range(72):  # ~5us of tiny MMs: flips the PE clock-gate early
            nc.tensor.matmul(wps[0:2, 0:2], ebias_b, ebias_b,
                             start=True, stop=True, skip_group_check=True)

        maskband = consts.tile([128, 256], BF16, name="maskband")
        nc.sync.dma_start(out=maskband, in_=mb_h[:, :])
        bias_qk = consts.tile([128, 8], F32, name="bias_qk")
        nc.sync.dma_start(out=bias_qk, in_=bqk_h[:, :])
        bias_kc = consts.tile([128, 4], F32, name="bias_kc")
        nc.sync.dma_start(out=bias_kc, in_=bkc_h[:, :])
        bvb = consts.tile([128, 512], F32, name="bvb")
        nc.gpsimd.dma_start(out=bvb, in_=_bc0(bv_h[:, :], 128))
        bvcb = consts.tile([128, 512], F32, name="bvcb")
        nc.gpsimd.dma_start(out=bvcb, in_=_bc0(bvc_h[:, :], 128))

        # ---- input loads, in consumption order (sync queue is FIFO) ----
        ctx_sb = wts.tile([128, 8, Tc], BF16, name="ctx_sb")
        nc.sync.dma_start(out=ctx_sb, in_=ctx_h[:, :, :])
        wkc_sb = wts.tile([128, 8, 512], BF16, name="wkc_sb")
        nc.sync.dma_start(out=wkc_sb, in_=wkc_h[:, :, :])
        wvc_sb = wts.tile([128, 8, 512], BF16, name="wvc_sb")
        nc.sync.dma_start(out=wvc_sb, in_=wvc_h[:, :, :])
        wv_sb = wts.tile([128, 8, 512], BF16, name="wv_sb")
        nc.sync.dma_start(out=wv_sb, in_=wv_h[:, :, :])
        x_sb = wts.tile([128, 8, T], BF16, name="x_sb")
        nc.sync.dma_start(out=x_sb, in_=x_h[:, :, :])
        wq_sb = wts.tile([128, 8, 512], BF16, name="wq_sb")
        nc.sync.dma_start(out=wq_sb, in_=wq_h[:, :, :])
        wk_sb = wts.tile([128, 8, 512], BF16, name="wk_sb")
        nc.sync.dma_start(out=wk_sb, in_=wk_h[:, :, :])
        wpj_sb = wts.tile([128, 4, NX], BF16, name="wpj_sb")
        nc.sync.dma_start(out=wpj_sb, in_=wpj_h[:, :, :])

        # ---- persistent activation tiles ----
        qT = [qkp.tile([128, T], BF16, name=f"qT{p}") for p in range(NPAIR)]
        kT = [qkp.tile([128, Tc + T], BF16, name=f"kT{p}") for p in range(NPAIR)]
        v_sb = [vtp.tile([128, 8, 65], BF16, name=f"v{c}") for c in range(8)]
        for c in range(8):
            nc.gpsimd.memset(v_sb[c][:, :, 64:65], 1.0)
        aT = [atp_sb.tile([128, T], BF16, name=f"aT{p}") for p in range(NPAIR)]
        tmp = [atp_sb.tile([65, 2, T], BF16, name=f"tmp{p}") for p in range(NPAIR)]

        # ---- ctx projections: kcT (per pair) + vc ----
        for f in range(NPAIR):
            pt = psg.tile([128, 512], F32, tag="gen", name=f"pkc{f}")
            for kc in range(8):
                nc.tensor.matmul(
                    pt[:, 0:Tc], wkc_sb[:, kc, 128 * f:128 * f + 128],
                    ctx_sb[:, kc, :], start=(kc == 0), stop=(kc == 7),
                    skip_group_check=True)
            nc.scalar.activation(
                out=kT[f][:, 0:Tc], in_=pt[:, 0:Tc], func=AF.Identity,
                bias=bias_kc[:, f:f + 1], scale=1.0)
        pt = psg.tile([128, 512], F32, tag="gen", name="pvc")
        for kc in range(8):
            nc.tensor.matmul(
                pt[:, 0:512], ctx_sb[:, kc, :], wvc_sb[:, kc, :],
                start=(kc == 0), stop=(kc == 7), skip_group_check=True)
        nc.vector.tensor_add(
            out=v_sb[0][:, :, 0:64],
            in0=pt[:, 0:512].rearrange("p (h d) -> p h d", h=8),
            in1=bvcb.rearrange("p (h d) -> p h d", h=8))

        # ---- v projection (natural layout) ----
        for tt in range(7):
            pt = psg.tile([128, 512], F32, tag="gen", name=f"pv{tt}")
            for kc in range(8):
                nc.tensor.matmul(
                    pt[:, 0:512], x_sb[:, kc, 128 * tt:128 * tt + 128],
                    wv_sb[:, kc, :], start=(kc == 0), stop=(kc == 7),
                    skip_group_check=True)
            nc.vector.tensor_add(
                out=v_sb[1 + tt][:, :, 0:64],
                in0=pt[:, 0:512].rearrange("p (h d) -> p h d", h=8),
                in1=bvb.rearrange("p (h d) -> p h d", h=8))

        # ---- qT / kT projections, as a generator for interleaving ----
        def gen_qk(p):
            for wsb, dest, dcol, bcol in ((wq_sb, qT[p], 0, p),
                                          (wk_sb, kT[p], Tc, 4 + p)):
                for ts, te in TCH:
                    pt = psg.tile([128, 512], F32, tag="gen",
                                  name=f"pqk{p}{dcol}{ts}")
                    for kc in range(8):
                        nc.tensor.matmul(
                            pt[:, 0:te - ts], wsb[:, kc, 128 * p:128 * p + 128],
                            x_sb[:, kc, ts:te], start=(kc == 0), stop=(kc == 7),
                            skip_group_check=True)
                        yield
                    nc.scalar.activation(
                        out=dest[:, dcol + ts:dcol + te], in_=pt[:, 0:te - ts],
                        func=AF.Identity, bias=bias_qk[:, bcol:bcol + 1],
                        scale=1.0)
                    yield

        # ---- out projection per sweep, as a generator ----
        def gen_outproj(si):
            ts, te, _ = SWEEPS[si]
            for of in range(8):
                pt = psg.tile([128, 512], F32, tag="gen", name=f"po{si}{of}")
                for kc in range(NPAIR):
                    nc.tensor.matmul(
                        pt[:, 0:te - ts], wpj_sb[:, kc, 128 * of:128 * of + 128],
                        aT[kc][:, ts:te], start=(kc == 0), stop=(kc == 3),
                        skip_group_check=True)
                    yield
                ob = outp.tile([128, 512], F32, tag="ob", name=f"ob{si}{of}")
                if si < 2:  # ACT is exp-busy during sweeps; idle at the tail
                    nc.vector.tensor_copy(out=ob[:, 0:te - ts], in_=pt[:, 0:te - ts])
                else:
                    nc.scalar.copy(out=ob[:, 0:te - ts], in_=pt[:, 0:te - ts])
                yield
                nc.sync.dma_start(out=out_h[128 * of:128 * of + 128, ts:te],
                                  in_=ob[:, 0:te - ts])
                yield

        fillers = deque()

        def drain(n):
            for _ in range(n):
                if not fillers:
                    return
                gen = fillers[0]
                try:
                    next(gen)
                except StopIteration:
                    fillers.popleft()

        # ---- attention for one (pair, sweep) ----
        def attention(p, si, slots_budget):
            ts, te, last_c = SWEEPS[si]
            chunks = [c for c in range(8) if _rect(c, ts, te) is not None]
            nch = len(chunks)
            at_ps = atps.tile([65, 2, 512], F32, tag="at", name=f"at{p}{si}")
            for c in chunks:
                cs, _ = _rect(c, ts, te)
                sc = scp.tile([128, 2, 512], F32, tag="sc", name=f"sc{p}{si}{c}")
                for hi in range(2):
                    nc.tensor.matmul(
                        sc[:, hi, cs - ts:te - ts],
                        kT[p][64 * hi:64 * hi + 64, 128 * c:128 * c + 128],
                        qT[p][64 * hi:64 * hi + 64, cs:te],
                        start=True, stop=True, tile_position=(64 * hi, 0))
                pb = probsp.tile([128, 2, 512], BF16, tag="pb", name=f"pb{p}{si}{c}")
                nc.scalar.activation(
                    out=pb[:, :, cs - ts:te - ts], in_=sc[:, :, cs - ts:te - ts],
                    func=AF.Exp,
                    bias=(ebias[:, 1:2] if c == 0 else ebias[:, 0:1]),
                    scale=0.125)
                for hi in range(2):
                    for s0, e0, mc in _band_pieces(c, ts, te):
                        nc.vector.tensor_mul(
                            out=pb[:, hi, s0 - ts:e0 - ts],
                            in0=pb[:, hi, s0 - ts:e0 - ts],
                            in1=maskband[:, mc:mc + (e0 - s0)])
                # PE fillers hide the exp+mask latency before this chunk's PV
                drain(-(-slots_budget // nch))
                for hi in range(2):
                    nc.tensor.matmul(
                        at_ps[0:65, hi, cs - ts:te - ts],
                        v_sb[c][:, 2 * p + hi, :],
                        pb[:, hi, cs - ts:te - ts],
                        start=(c == 0), stop=(c == last_c),
                        skip_group_check=True)
            # evacuate aT(+denominator row 64) to SBUF as bf16
            nc.vector.tensor_copy(out=tmp[p][0:65, :, ts:te],
                                  in_=at_ps[0:65, :, 0:te - ts])

        # ---- per-(pair, sweep) normalization + aT assembly ----
        def assemble(p, si):
            ts, te, _ = SWEEPS[si]
            ncols = (te - ts) // 128
            dm = misc.tile([128, 2, 4], BF16, tag="dm", name=f"dm{p}{si}")
            rdm = misc.tile([128, 2, 4], F32, tag="rdm", name=f"rdm{p}{si}")
            rscr = dramp.tile([2, T], F32, tag="rscr", name=f"rscr{p}{si}")
            rbc = misc.tile([128, T], BF16, tag="rbc", name=f"rbc{p}{si}")
            for hi in range(2):
                nc.sync.dma_start(out=dm[:, hi, 0:ncols],
                                  in_=tmp[p][64:65, hi, ts:te])
            nc.vector.reciprocal(out=rdm[:, :, 0:ncols], in_=dm[:, :, 0:ncols])
            for hi in range(2):
                nc.sync.dma_start(out=rscr[hi:hi + 1, ts:te],
                                  in_=rdm[:, hi, 0:ncols])
            for hi in range(2):
                nc.gpsimd.dma_start(
                    out=rbc[64 * hi:64 * hi + 64, ts:te],
                    in_=_bc0(rscr[hi:hi + 1, ts:te], 64))
            nc.gpsimd.dma_start(out=aT[p][0:64, ts:te], in_=tmp[p][0:64, 0, ts:te])
            nc.gpsimd.dma_start(out=aT[p][64:128, ts:te], in_=tmp[p][0:64, 1, ts:te])
            nc.vector.tensor_mul(out=aT[p][:, ts:te], in0=aT[p][:, ts:te],
                                 in1=rbc[:, ts:te])

        # ---- phase program ----
        for _ in gen_qk(0):  # emit qk(0) directly
            pass
        for si in range(3):
            for p in range(NPAIR):
                if si == 0:
                    if p + 1 < NPAIR:
                        fillers.append(gen_qk(p + 1))
                    budget = 36 if p + 1 < NPAIR else 0
                else:
                    budget = 12
                attention(p, si, slots_budget=budget)
                assemble(p, si)
            drain(10**6)
            if si < 2:
                fillers.append(gen_outproj(si))
        drain(10**6)
        for _ in gen_outproj(2):  # small tail: tokens 768:896
            pass

    if not nc.is_finalized():
        nc.finalize()
    return nc


_NC_CACHE = {}


def _get_nc():
    if "nc" not in _NC_CACHE:
        _NC_CACHE["nc"] = build_nc()
    return _NC_CACHE["nc"]


def _pack128(v):
    """[128*n] -> [128, n] with [p, f] = v[128*f + p]."""
    n = v.shape[0] // 128
    return np.ascontiguousarray(v.reshape(n, 128).T)


def _perm(w2d, dtype):
    """[128*kc, f] -> [128, kc, f] contiguous (SBUF layout built on host)."""
    kc = w2d.shape[0] // 128
    return np.ascontiguousarray(
        w2d.reshape(kc, 128, w2d.shape[1]).transpose(1, 0, 2).astype(dtype))


def make_in_maps(inputs):
    bf16 = ml_dtypes.bfloat16
    x = np.asarray(inputs["x"], np.float32)
    ctx_seq = np.asarray(inputs["context_seq"], np.float32)
    w_ref = np.asarray(inputs["w_ref"], np.float32)
    b_ref = np.asarray(inputs["b_ref"], np.float32)
    w_attn = np.asarray(inputs["w_attn"], np.float32)
    b_attn = np.asarray(inputs["b_attn"], np.float32)
    w_proj = np.asarray(inputs["w_proj"], np.float32)

    # mask band constant: cols 0-127 causal (1 where q>=p), cols 128-255
    # anti-diagonal (0 where q==p else 1)
    qq = np.arange(128)[None, :]
    pp = np.arange(128)[:, None]
    mband = np.concatenate([(qq >= pp), (qq != pp)], axis=1).astype(bf16)
    mband = np.ascontiguousarray(mband)

    in_maps = []
    for b in range(4):
        xr = _perm(x[b].T, bf16)
        ctxr = _perm(ctx_seq[b].T, bf16)
        for g in range(2):
            sl = slice(512 * g, 512 * g + 512)
            in_maps.append(dict(
                xr=xr,
                ctxr=ctxr,
                w_q=_perm(w_attn[:, 0 * NX:1 * NX][:, sl], bf16),
                w_k=_perm(w_attn[:, 1 * NX:2 * NX][:, sl], bf16),
                w_v=_perm(w_attn[:, 2 * NX:3 * NX][:, sl], bf16),
                w_kc=_perm(w_ref[:, 0 * NX:1 * NX][:, sl], bf16),
                w_vc=_perm(w_ref[:, 1 * NX:2 * NX][:, sl], bf16),
                w_pj=_perm(w_proj[sl, :], bf16),
                b_qk=_pack128(np.concatenate([b_attn[0 * NX:1 * NX][sl],
                                              b_attn[1 * NX:2 * NX][sl]])),
                b_kc=_pack128(b_ref[0 * NX:1 * NX][sl]),
                b_v=np.ascontiguousarray(b_attn[2 * NX:3 * NX][sl].reshape(1, 512)),
                b_vc=np.ascontiguousarray(b_ref[1 * NX:2 * NX][sl].reshape(1, 512)),
                mband=mband,
            ))
    return in_maps


def kernel(**inputs):
    b_proj = np.asarray(inputs["b_proj"], np.float32)
    in_maps = make_in_maps(inputs)
    nc = _get_nc()
    res = run_bass_kernel_spmd(nc, in_maps, core_ids=list(range(8)),
                               trace=os.environ.get("COCON_TRACE", "") == "1")
    outs = res.results
    out = np.empty((4, T, NX), np.float32)
    for b in range(4):
        acc = outs[2 * b]["outT"] + outs[2 * b + 1]["outT"]  # [1024, 896]
        out[b] = acc.T + b_proj[None, :]
    if res.exec_time_ns is not None:
        kernel.last_exec_time_ns = res.exec_time_ns
    return out


kernel.last_exec_time_ns = None


# revision 7
# speedup vs baseline: 1.0516x; 1.0516x over previous
"""Trainium2 Bass kernel for nn_CoconAttention (dense transformer attention block).

Sharding: 8 cores = 4 batches x 2 head-groups (8 heads each). Each core gets
pre-permuted bf16 inputs ([128, kc, ...] SBUF layouts built on host so DMAs are
fat + contiguous), computes its partial output outT [1024, 896] (transposed,
pre-b_proj), and the host sums head-group pairs + transposes.

v4 (per core, H=8 heads, Dh=64, T=896, Tc=128, S=1024):
  all matmul operands bf16 (psum accumulation fp32); exp on ACT; masks on DVE.
  Software-pipelined emission over three token sweeps (0:512, 512:768,
  768:896): attention(p, sweep0) interleaves qk-proj(p+1) MMs into the PE
  stream; attention(p, sweep k>0) interleaves out-proj(sweep k-1) MMs and the
  previous pair's normalization.  Denominators for a sweep are final at its
  end (causal window j <= i+128).  Normalization is DMA-free: PV's stationary
  carries a leading ones-column (denominator lands in psum row 0); a K=1
  ones-matmul broadcasts it across partitions, DVE reciprocates, a
  shifted-identity matmul splits the two heads onto partitions 0-64/64-128,
  and one DVE multiply writes normalized bf16 aT.
"""
import os
import sys
from collections import deque

import numpy as np
import ml_dtypes

try:
    import concourse.bass as bass
except ImportError:  # fresh grading dir: fall back to the repo location
    sys.path.insert(0, "/opt/trn_rl_repo")
    import concourse.bass as bass
import concourse.bacc as bacc

import concourse.tile as tile
from concourse import mybir
from concourse.bass_utils import run_bass_kernel_spmd
from contextlib import ExitStack

F32 = mybir.dt.float32
BF16 = mybir.dt.bfloat16
AF = mybir.ActivationFunctionType
ALU = mybir.AluOpType

T, Tc, NX = 896, 128, 1024
TCH = ((0, 512), (512, 896))  # qk-proj tok chunks (psum-width limited)
SWEEPS = ((0, 512, 4), (512, 768, 6), (768, 896, 7))  # (ts, te, last_c)
NPAIR = 4  # head pairs per core


def _bc0(ap, n):
    """Partition-broadcast read AP: [1, ...] -> [n, ...] with partition step 0."""
    return bass.AP(tensor=ap.tensor, offset=ap.offset, ap=[[0, n]] + list(ap.ap[1:]))


def _rect(c, ts, te):
    """Live (unmasked) column range of scores chunk c within tok range [ts,te)."""
    cs = max(max(0, 128 * (c - 1)), ts)
    return None if cs >= te else (cs, te)


def _band_pieces(c, ts, te):
    """Mask applications for chunk c in [ts,te): (s0, e0, mask_col_offset)."""
    if c == 0:
        bs, be, moff, borig = 0, 128, 128, 0  # diag half only
    elif c <= 6:
        bs = 128 * (c - 1)
        be, moff, borig = bs + 256, 0, bs  # causal(128) + diag(128)
    else:
        bs, be, moff, borig = 768, 896, 0, 768  # causal half only
    s0, e0 = max(bs, ts), min(be, te)
    if s0 >= e0:
        return []
    return [(s0, e0, moff + (s0 - borig))]


def build_nc():
    nc = bacc.Bacc("TRN2", target_bir_lowering=False)

    x_h = nc.dram_tensor("xr", [128, 8, T], BF16, kind="ExternalInput")
    ctx_h = nc.dram_tensor("ctxr", [128, 8, Tc], BF16, kind="ExternalInput")
    wq_h = nc.dram_tensor("w_q", [128, 8, 512], BF16, kind="ExternalInput")
    wk_h = nc.dram_tensor("w_k", [128, 8, 512], BF16, kind="ExternalInput")
    wv_h = nc.dram_tensor("w_v", [128, 8, 512], BF16, kind="ExternalInput")
    wkc_h = nc.dram_tensor("w_kc", [128, 8, 512], BF16, kind="ExternalInput")
    wvc_h = nc.dram_tensor("w_vc", [128, 8, 512], BF16, kind="ExternalInput")
    wpj_h = nc.dram_tensor("w_pj", [128, 4, NX], BF16, kind="ExternalInput")
    bqk_h = nc.dram_tensor("b_qk", [128, 8], F32, kind="ExternalInput")
    bkc_h = nc.dram_tensor("b_kc", [128, 4], F32, kind="ExternalInput")
    bv_h = nc.dram_tensor("b_v", [1, 512], F32, kind="ExternalInput")
    bvc_h = nc.dram_tensor("b_vc", [1, 512], F32, kind="ExternalInput")
    mb_h = nc.dram_tensor("mband", [128, 256], BF16, kind="ExternalInput")
    esh_h = nc.dram_tensor("eshift", [65, 64], BF16, kind="ExternalInput")
    out_h = nc.dram_tensor("outT", [NX, T], F32, kind="ExternalOutput")

    with tile.TileContext(nc) as tc, ExitStack() as top:
        consts = top.enter_context(tc.tile_pool(name="consts", bufs=1))
        qkp = top.enter_context(tc.tile_pool(name="qkp", bufs=1))
        vtp = top.enter_context(tc.tile_pool(name="vtp", bufs=1))
        atp_sb = top.enter_context(tc.tile_pool(name="atp_sb", bufs=1))
        wts = top.enter_context(tc.tile_pool(name="wts", bufs=1))
        misc = top.enter_context(tc.tile_pool(name="misc", bufs=2))
        probsp = top.enter_context(tc.tile_pool(name="probsp", bufs=4))
        outp = top.enter_context(tc.tile_pool(name="outp", bufs=2))
        psg = top.enter_context(tc.tile_pool(name="psg", bufs=2, space="PSUM"))
        scp = top.enter_context(tc.tile_pool(name="scp", bufs=2, space="PSUM"))
        atps = top.enter_context(tc.tile_pool(name="atps", bufs=1, space="PSUM"))

        # ---- tiny consts + ACT exp table warmup + PE HAM warmup ----
        ebias = consts.tile([128, 2], F32, name="ebias")  # [0]=0, [1]=ctx bias -2
        nc.vector.memset(ebias[:, 0:1], 0.0)
        nc.vector.memset(ebias[:, 1:2], -2.0)
        warm = consts.tile([128, 2], F32, name="warm")
        nc.scalar.activation(out=warm, in_=ebias, func=AF.Exp, scale=1.0)
        ebias_b = consts.tile([128, 2], BF16, name="ebias_b")
        nc.vector.tensor_copy(out=ebias_b, in_=ebias)
        ones_c = consts.tile([1, 64], BF16, name="ones_c")
        nc.gpsimd.memset(ones_c, 1.0)
        wps = psg.tile([128, 512], F32, tag="gen", name="warmps")
        for _ in range(160):  # ~8-12us of tiny MMs: keeps the PE clock-gate
            nc.tensor.matmul(wps[0:2, 0:2], ebias_b, ebias_b,  # warm during DMA
                             start=True, stop=True, skip_group_check=True)

        bias_qk = consts.tile([128, 8], F32, name="bias_qk")
        nc.sync.dma_start(out=bias_qk, in_=bqk_h[:, :])
        bias_kc = consts.tile([128, 4], F32, name="bias_kc")
        nc.sync.dma_start(out=bias_kc, in_=bkc_h[:, :])
        bvb = consts.tile([128, 512], F32, name="bvb")
        nc.gpsimd.dma_start(out=bvb, in_=_bc0(bv_h[:, :], 128))
        bvcb = consts.tile([128, 512], F32, name="bvcb")
        nc.gpsimd.dma_start(out=bvcb, in_=_bc0(bvc_h[:, :], 128))

        # ---- input loads, in consumption order (sync queue is FIFO) ----
        ctx_sb = wts.tile([128, 8, Tc], BF16, name="ctx_sb")
        nc.sync.dma_start(out=ctx_sb, in_=ctx_h[:, :, :])
        wkc_sb = wts.tile([128, 8, 512], BF16, name="wkc_sb")
        nc.sync.dma_start(out=wkc_sb, in_=wkc_h[:, :, :])
        wvc_sb = wts.tile([128, 8, 512], BF16, name="wvc_sb")
        nc.sync.dma_start(out=wvc_sb, in_=wvc_h[:, :, :])
        wv_sb = wts.tile([128, 8, 512], BF16, name="wv_sb")
        nc.sync.dma_start(out=wv_sb, in_=wv_h[:, :, :])
        x_sb = wts.tile([128, 8, T], BF16, name="x_sb")
        nc.sync.dma_start(out=x_sb, in_=x_h[:, :, :])
        wq_sb = wts.tile([128, 8, 512], BF16, name="wq_sb")
        nc.sync.dma_start(out=wq_sb, in_=wq_h[:, :, :])
        wk_sb = wts.tile([128, 8, 512], BF16, name="wk_sb")
        nc.sync.dma_start(out=wk_sb, in_=wk_h[:, :, :])
        maskband = consts.tile([128, 256], BF16, name="maskband")
        nc.sync.dma_start(out=maskband, in_=mb_h[:, :])
        eshift = consts.tile([65, 64], BF16, name="eshift")
        nc.sync.dma_start(out=eshift, in_=esh_h[:, :])
        wpj_sb = wts.tile([128, 4, NX], BF16, name="wpj_sb")
        nc.sync.dma_start(out=wpj_sb, in_=wpj_h[:, :, :])

        # ---- persistent activation tiles ----
        qT = [qkp.tile([128, T], BF16, name=f"qT{p}") for p in range(NPAIR)]
        kT = [qkp.tile([128, Tc + T], BF16, name=f"kT{p}") for p in range(NPAIR)]
        # v_aug column 0 is ones -> PV psum row 0 = softmax denominator
        v_sb = [vtp.tile([128, 8, 65], BF16, name=f"v{c}") for c in range(8)]
        for c in range(8):
            nc.gpsimd.memset(v_sb[c][:, :, 0:1], 1.0)
        aT = [atp_sb.tile([128, T], BF16, name=f"aT{p}") for p in range(NPAIR)]
        tmp = [atp_sb.tile([65, 2, T], BF16, name=f"tmp{p}") for p in range(NPAIR)]

        # ---- ctx projections: kcT (per pair) + vc ----
        for f in range(NPAIR):
            pt = psg.tile([128, 512], F32, tag="gen", name=f"pkc{f}")
            for kc in range(8):
                nc.tensor.matmul(
                    pt[:, 0:Tc], wkc_sb[:, kc, 128 * f:128 * f + 128],
                    ctx_sb[:, kc, :], start=(kc == 0), stop=(kc == 7),
                    skip_group_check=True)
            nc.scalar.activation(
                out=kT[f][:, 0:Tc], in_=pt[:, 0:Tc], func=AF.Identity,
                bias=bias_kc[:, f:f + 1], scale=1.0)
        pt = psg.tile([128, 512], F32, tag="gen", name="pvc")
        for kc in range(8):
            nc.tensor.matmul(
                pt[:, 0:512], ctx_sb[:, kc, :], wvc_sb[:, kc, :],
                start=(kc == 0), stop=(kc == 7), skip_group_check=True)
        nc.vector.tensor_add(
            out=v_sb[0][:, :, 1:65],
            in0=pt[:, 0:512].rearrange("p (h d) -> p h d", h=8),
            in1=bvcb.rearrange("p (h d) -> p h d", h=8))

        # ---- v projection (natural layout) ----
        for tt in range(7):
            pt = psg.tile([128, 512], F32, tag="gen", name=f"pv{tt}")
            for kc in range(8):
                nc.tensor.matmul(
                    pt[:, 0:512], x_sb[:, kc, 128 * tt:128 * tt + 128],
                    wv_sb[:, kc, :], start=(kc == 0), stop=(kc == 7),
                    skip_group_check=True)
            nc.vector.tensor_add(
                out=v_sb[1 + tt][:, :, 1:65],
                in0=pt[:, 0:512].rearrange("p (h d) -> p h d", h=8),
                in1=bvb.rearrange("p (h d) -> p h d", h=8))

        # ---- qT / kT projections, as a generator for interleaving ----
        def gen_qk(p):
            for wsb, dest, dcol, bcol in ((wq_sb, qT[p], 0, p),
                                          (wk_sb, kT[p], Tc, 4 + p)):
                for ts, te in TCH:
                    pt = psg.tile([128, 512], F32, tag="gen",
                                  name=f"pqk{p}{dcol}{ts}")
                    for kc in range(8):
                        nc.tensor.matmul(
                            pt[:, 0:te - ts], wsb[:, kc, 128 * p:128 * p + 128],
                            x_sb[:, kc, ts:te], start=(kc == 0), stop=(kc == 7),
                            skip_group_check=True)
                        yield
                    nc.scalar.activation(
                        out=dest[:, dcol + ts:dcol + te], in_=pt[:, 0:te - ts],
                        func=AF.Identity, bias=bias_qk[:, bcol:bcol + 1],
                        scale=1.0)
                    yield

        # ---- out projection per sweep, as a generator ----
        def gen_outproj(si):
            ts, te, _ = SWEEPS[si]
            for of in range(8):
                pt = psg.tile([128, 512], F32, tag="gen", name=f"po{si}{of}")
                for kc in range(NPAIR):
                    nc.tensor.matmul(
                        pt[:, 0:te - ts], wpj_sb[:, kc, 128 * of:128 * of + 128],
                        aT[kc][:, ts:te], start=(kc == 0), stop=(kc == 3),
                        skip_group_check=True)
                    yield
                ob = outp.tile([128, 512], F32, tag="ob", name=f"ob{si}{of}")
                if si < 2:  # ACT is exp-busy mid-kernel (gpsimd can't read PSUM)
                    nc.vector.tensor_copy(out=ob[:, 0:te - ts], in_=pt[:, 0:te - ts])
                else:
                    nc.scalar.copy(out=ob[:, 0:te - ts], in_=pt[:, 0:te - ts])
                yield
                nc.sync.dma_start(out=out_h[128 * of:128 * of + 128, ts:te],
                                  in_=ob[:, 0:te - ts])
                yield

        fillers = deque()

        def drain(n):
            for _ in range(n):
                if not fillers:
                    return
                gen = fillers[0]
                try:
                    next(gen)
                except StopIteration:
                    fillers.popleft()

        # ---- attention for one (pair, sweep) ----
        def attention(p, si, slots_budget):
            ts, te, last_c = SWEEPS[si]
            chunks = [c for c in range(8) if _rect(c, ts, te) is not None]
            nch = len(chunks)
            at_ps = atps.tile([65, 2, 512], F32, tag="at", name=f"at{p}{si}")
            for c in chunks:
                cs, _ = _rect(c, ts, te)
                sc = scp.tile([128, 2, 512], F32, tag="sc", name=f"sc{p}{si}{c}")
                for hi in range(2):
                    nc.tensor.matmul(
                        sc[:, hi, cs - ts:te - ts],
                        kT[p][64 * hi:64 * hi + 64, 128 * c:128 * c + 128],
                        qT[p][64 * hi:64 * hi + 64, cs:te],
                        start=True, stop=True, tile_position=(64 * hi, 0))
                pb = probsp.tile([128, 2, 512], BF16, tag="pb", name=f"pb{p}{si}{c}")
                nc.scalar.activation(
                    out=pb[:, :, cs - ts:te - ts], in_=sc[:, :, cs - ts:te - ts],
                    func=AF.Exp,
                    bias=(ebias[:, 1:2] if c == 0 else ebias[:, 0:1]),
                    scale=0.125)
                for hi in range(2):
                    for s0, e0, mc in _band_pieces(c, ts, te):
                        nc.vector.tensor_mul(
                            out=pb[:, hi, s0 - ts:e0 - ts],
                            in0=pb[:, hi, s0 - ts:e0 - ts],
                            in1=maskband[:, mc:mc + (e0 - s0)])
                # PE fillers hide the exp+mask latency before this chunk's PV
                drain(-(-slots_budget // nch))
                for hi in range(2):
                    nc.tensor.matmul(
                        at_ps[0:65, hi, cs - ts:te - ts],
                        v_sb[c][:, 2 * p + hi, :],
                        pb[:, hi, cs - ts:te - ts],
                        start=(c == 0), stop=(c == last_c),
                        skip_group_check=True)
            # evacuate aT(+denominator row 0) to SBUF as bf16
            nc.vector.tensor_copy(out=tmp[p][0:65, :, ts:te],
                                  in_=at_ps[0:65, :, 0:te - ts])

        # ---- per-(pair, sweep) normalization + aT assembly: all on-chip ----
        def gen_asm(p, si):
            ts, te, _ = SWEEPS[si]
            w = te - ts
            # broadcast denominators (psum row 0 of each head) to di[64hi..]
            di = psg.tile([128, 512], F32, tag="gen", name=f"di{p}{si}")
            for hi in range(2):
                nc.tensor.matmul(
                    di[64 * hi:64 * hi + 64, 0:w], ones_c,
                    tmp[p][0:1, hi, ts:te], start=True, stop=True,
                    tile_position=(0, 64 * hi), skip_group_check=True)
                yield
            rbc = misc.tile([128, 512], F32, tag="rbc", name=f"rbc{p}{si}")
            nc.vector.reciprocal(out=rbc[:, 0:w], in_=di[:, 0:w])
            yield
            # shift heads onto partitions 0-64 / 64-128 (rows 1-64 of tmp)
            aps = psg.tile([128, 512], F32, tag="gen", name=f"aps{p}{si}")
            for hi in range(2):
                nc.tensor.matmul(
                    aps[64 * hi:64 * hi + 64, 0:w], eshift,
                    tmp[p][0:65, hi, ts:te], start=True, stop=True,
                    tile_position=(0, 64 * hi), skip_group_check=True)
                yield
            nc.vector.tensor_mul(out=aT[p][:, ts:te], in0=aps[:, 0:w],
                                 in1=rbc[:, 0:w])
            yield

        # ---- phase program ----
        for _ in gen_qk(0):  # emit qk(0) directly
            pass
        for si in range(3):
            for p in range(NPAIR):
                if si == 0:
                    if p + 1 < NPAIR:
                        fillers.append(gen_qk(p + 1))
                    budget = 44 if p + 1 < NPAIR else 8
                else:
                    budget = 21
                attention(p, si, slots_budget=budget)
                fillers.appendleft(gen_asm(p, si))
            drain(10**6)
            if si < 2:
                fillers.append(gen_outproj(si))
        drain(10**6)
        for _ in gen_outproj(2):  # small tail: tokens 768:896
            pass

    if not nc.is_finalized():
        nc.finalize()
    return nc


_NC_CACHE = {}


def _get_nc():
    if "nc" not in _NC_CACHE:
        _NC_CACHE["nc"] = build_nc()
    return _NC_CACHE["nc"]


def _pack128(v):
    """[128*n] -> [128, n] with [p, f] = v[128*f + p]."""
    n = v.shape[0] // 128
    return np.ascontiguousarray(v.reshape(n, 128).T)


def _perm(w2d, dtype):
    """[128*kc, f] -> [128, kc, f] contiguous (SBUF layout built on host)."""
    kc = w2d.shape[0] // 128
    return np.ascontiguousarray(
        w2d.reshape(kc, 128, w2d.shape[1]).transpose(1, 0, 2).astype(dtype))


def make_in_maps(inputs):
    bf16 = ml_dtypes.bfloat16
    x = np.asarray(inputs["x"], np.float32)
    ctx_seq = np.asarray(inputs["context_seq"], np.float32)
    w_ref = np.asarray(inputs["w_ref"], np.float32)
    b_ref = np.asarray(inputs["b_ref"], np.float32)
    w_attn = np.asarray(inputs["w_attn"], np.float32)
    b_attn = np.asarray(inputs["b_attn"], np.float32)
    w_proj = np.asarray(inputs["w_proj"], np.float32)

    # mask band constant: cols 0-127 causal (1 where q>=p), cols 128-255
    # anti-diagonal (0 where q==p else 1)
    qq = np.arange(128)[None, :]
    pp = np.arange(128)[:, None]
    mband = np.concatenate([(qq >= pp), (qq != pp)], axis=1).astype(bf16)
    mband = np.ascontiguousarray(mband)
    eshift = np.ascontiguousarray(np.eye(65, 64, k=-1).astype(bf16))

    in_maps = []
    for b in range(4):
        xr = _perm(x[b].T, bf16)
        ctxr = _perm(ctx_seq[b].T, bf16)
        for g in range(2):
            sl = slice(512 * g, 512 * g + 512)
            in_maps.append(dict(
                xr=xr,
                ctxr=ctxr,
                w_q=_perm(w_attn[:, 0 * NX:1 * NX][:, sl], bf16),
                w_k=_perm(w_attn[:, 1 * NX:2 * NX][:, sl], bf16),
                w_v=_perm(w_attn[:, 2 * NX:3 * NX][:, sl], bf16),
                w_kc=_perm(w_ref[:, 0 * NX:1 * NX][:, sl], bf16),
                w_vc=_perm(w_ref[:, 1 * NX:2 * NX][:, sl], bf16),
                w_pj=_perm(w_proj[sl, :], bf16),
                b_qk=_pack128(np.concatenate([b_attn[0 * NX:1 * NX][sl],
                                              b_attn[1 * NX:2 * NX][sl]])),
                b_kc=_pack128(b_ref[0 * NX:1 * NX][sl]),
                b_v=np.ascontiguousarray(b_attn[2 * NX:3 * NX][sl].reshape(1, 512)),
                b_vc=np.ascontiguousarray(b_ref[1 * NX:2 * NX][sl].reshape(1, 512)),
                mband=mband,
                eshift=eshift,
            ))
    return in_maps


def kernel(**inputs):
    b_proj = np.asarray(inputs["b_proj"], np.float32)
    in_maps = make_in_maps(inputs)
    nc = _get_nc()
    res = run_bass_kernel_spmd(nc, in_maps, core_ids=list(range(8)),
                               trace=os.environ.get("COCON_TRACE", "") == "1")
    outs = res.results
    out = np.empty((4, T, NX), np.float32)
    for b in range(4):
        acc = outs[2 * b]["outT"] + outs[2 * b + 1]["outT"]  # [1024, 896]
        out[b] = acc.T + b_proj[None, :]
    if res.exec_time_ns is not None:
        kernel.last_exec_time_ns = res.exec_time_ns
    return out


kernel.last_exec_time_ns = None


# revision 8
# speedup vs baseline: 1.3033x; 1.2394x over previous
"""Trainium2 Bass kernel for nn_CoconAttention (dense transformer attention block).

Sharding: 8 cores = 4 batches x 2 head-groups (8 heads each). Each core gets
pre-permuted bf16 inputs ([128, kc, ...] SBUF layouts built on host so DMAs are
fat + contiguous), computes its partial output outT [1024, 896] (transposed,
pre-b_proj), and the host sums head-group pairs + transposes.

v4 (per core, H=8 heads, Dh=64, T=896, Tc=128, S=1024):
  all matmul operands bf16 (psum accumulation fp32); exp on ACT; masks on DVE.
  Software-pipelined emission over three token sweeps (0:512, 512:768,
  768:896): attention(p, sweep0) interleaves qk-proj(p+1) MMs into the PE
  stream; attention(p, sweep k>0) interleaves out-proj(sweep k-1) MMs and the
  previous pair's normalization.  Denominators for a sweep are final at its
  end (causal window j <= i+128).  Normalization is DMA-free: PV's stationary
  carries a leading ones-column (denominator lands in psum row 0); a K=1
  ones-matmul broadcasts it across partitions, DVE reciprocates, a
  shifted-identity matmul splits the two heads onto partitions 0-64/64-128,
  and one DVE multiply writes normalized bf16 aT.
"""
import os
import sys
from collections import deque

import numpy as np
import ml_dtypes

try:
    import concourse.bass as bass
except ImportError:  # fresh grading dir: fall back to the repo location
    sys.path.insert(0, "/opt/trn_rl_repo")
    import concourse.bass as bass
import concourse.bacc as bacc

import concourse.tile as tile
from concourse import mybir
from concourse.bass_utils import run_bass_kernel_spmd
from contextlib import ExitStack

F32 = mybir.dt.float32
BF16 = mybir.dt.bfloat16
AF = mybir.ActivationFunctionType
ALU = mybir.AluOpType

T, Tc, NX = 896, 128, 1024
TCH = ((0, 512), (512, 896))  # qk-proj tok chunks (psum-width limited)
SWEEPS = ((0, 512, 4), (512, 896, 7))  # (ts, te, last_c)
NPAIR = 4  # head pairs per core


def _bc0(ap, n):
    """Partition-broadcast read AP: [1, ...] -> [n, ...] with partition step 0."""
    return bass.AP(tensor=ap.tensor, offset=ap.offset, ap=[[0, n]] + list(ap.ap[1:]))


def _rect(c, ts, te):
    """Live (unmasked) column range of scores chunk c within tok range [ts,te)."""
    cs = max(max(0, 128 * (c - 1)), ts)
    return None if cs >= te else (cs, te)


def _band_pieces(c, ts, te):
    """Mask applications for chunk c in [ts,te): (s0, e0, mask_col_offset)."""
    if c == 0:
        bs, be, moff, borig = 0, 128, 128, 0  # diag half only
    elif c <= 6:
        bs = 128 * (c - 1)
        be, moff, borig = bs + 256, 0, bs  # causal(128) + diag(128)
    else:
        bs, be, moff, borig = 768, 896, 0, 768  # causal half only
    s0, e0 = max(bs, ts), min(be, te)
    if s0 >= e0:
        return []
    return [(s0, e0, moff + (s0 - borig))]


def build_nc():
    nc = bacc.Bacc("TRN2", target_bir_lowering=False)

    x_h = nc.dram_tensor("xr", [128, 8, T], BF16, kind="ExternalInput")
    ctx_h = nc.dram_tensor("ctxr", [128, 8, Tc], BF16, kind="ExternalInput")
    wq_h = nc.dram_tensor("w_q", [128, 8, 512], BF16, kind="ExternalInput")
    wk_h = nc.dram_tensor("w_k", [128, 8, 512], BF16, kind="ExternalInput")
    wv_h = nc.dram_tensor("w_v", [128, 8, 512], BF16, kind="ExternalInput")
    wkc_h = nc.dram_tensor("w_kc", [128, 8, 512], BF16, kind="ExternalInput")
    wvc_h = nc.dram_tensor("w_vc", [128, 8, 512], BF16, kind="ExternalInput")
    wpj_h = nc.dram_tensor("w_pj", [128, 4, NX], BF16, kind="ExternalInput")
    bqk_h = nc.dram_tensor("b_qk", [128, 8], F32, kind="ExternalInput")
    bkc_h = nc.dram_tensor("b_kc", [128, 4], F32, kind="ExternalInput")
    bv_h = nc.dram_tensor("b_v", [1, 512], F32, kind="ExternalInput")
    bvc_h = nc.dram_tensor("b_vc", [1, 512], F32, kind="ExternalInput")
    mb_h = nc.dram_tensor("mband", [128, 256], BF16, kind="ExternalInput")
    esh_h = nc.dram_tensor("eshift", [65, 64], BF16, kind="ExternalInput")
    out_h = nc.dram_tensor("outT", [NX, T], F32, kind="ExternalOutput")

    with tile.TileContext(nc) as tc, ExitStack() as top:
        consts = top.enter_context(tc.tile_pool(name="consts", bufs=1))
        qkp = top.enter_context(tc.tile_pool(name="qkp", bufs=1))
        vtp = top.enter_context(tc.tile_pool(name="vtp", bufs=1))
        atp_sb = top.enter_context(tc.tile_pool(name="atp_sb", bufs=1))
        wts = top.enter_context(tc.tile_pool(name="wts", bufs=1))
        misc = top.enter_context(tc.tile_pool(name="misc", bufs=2))
        probsp = top.enter_context(tc.tile_pool(name="probsp", bufs=4))
        outp = top.enter_context(tc.tile_pool(name="outp", bufs=2))
        psg = top.enter_context(tc.tile_pool(name="psg", bufs=2, space="PSUM"))
        scp = top.enter_context(tc.tile_pool(name="scp", bufs=2, space="PSUM"))
        atps = top.enter_context(tc.tile_pool(name="atps", bufs=1, space="PSUM"))

        # ---- tiny consts + ACT exp table warmup + PE HAM warmup ----
        ebias = consts.tile([128, 2], F32, name="ebias")  # [0]=0, [1]=ctx bias -2
        nc.vector.memset(ebias[:, 0:1], 0.0)
        nc.vector.memset(ebias[:, 1:2], -2.0)
        warm = consts.tile([128, 2], F32, name="warm")
        nc.scalar.activation(out=warm, in_=ebias, func=AF.Exp, scale=1.0)
        ebias_b = consts.tile([128, 2], BF16, name="ebias_b")
        nc.vector.tensor_copy(out=ebias_b, in_=ebias)
        ones_c = consts.tile([1, 64], BF16, name="ones_c")
        nc.gpsimd.memset(ones_c, 1.0)
        wps = psg.tile([128, 512], F32, tag="gen", name="warmps")
        for _ in range(250):  # ~8-12us of tiny MMs: keeps the PE clock-gate
            nc.tensor.matmul(wps[0:2, 0:2], ebias_b, ebias_b,  # warm during DMA
                             start=True, stop=True, skip_group_check=True)

        bias_qk = consts.tile([128, 8], F32, name="bias_qk")
        nc.sync.dma_start(out=bias_qk, in_=bqk_h[:, :])
        bias_kc = consts.tile([128, 4], F32, name="bias_kc")
        nc.sync.dma_start(out=bias_kc, in_=bkc_h[:, :])
        bvb = consts.tile([128, 512], F32, name="bvb")
        nc.gpsimd.dma_start(out=bvb, in_=_bc0(bv_h[:, :], 128))
        bvcb = consts.tile([128, 512], F32, name="bvcb")
        nc.gpsimd.dma_start(out=bvcb, in_=_bc0(bvc_h[:, :], 128))

        # ---- input loads, in consumption order (sync queue is FIFO) ----
        ctx_sb = wts.tile([128, 8, Tc], BF16, name="ctx_sb")
        nc.sync.dma_start(out=ctx_sb, in_=ctx_h[:, :, :])
        wkc_sb = wts.tile([128, 8, 512], BF16, name="wkc_sb")
        nc.sync.dma_start(out=wkc_sb, in_=wkc_h[:, :, :])
        wvc_sb = wts.tile([128, 8, 512], BF16, name="wvc_sb")
        nc.sync.dma_start(out=wvc_sb, in_=wvc_h[:, :, :])
        wv_sb = wts.tile([128, 8, 512], BF16, name="wv_sb")
        nc.sync.dma_start(out=wv_sb, in_=wv_h[:, :, :])
        x_sb = wts.tile([128, 8, T], BF16, name="x_sb")
        nc.sync.dma_start(out=x_sb, in_=x_h[:, :, :])
        wq_sb = wts.tile([128, 8, 512], BF16, name="wq_sb")
        nc.sync.dma_start(out=wq_sb, in_=wq_h[:, :, :])
        wk_sb = wts.tile([128, 8, 512], BF16, name="wk_sb")
        nc.sync.dma_start(out=wk_sb, in_=wk_h[:, :, :])
        maskband = consts.tile([128, 256], BF16, name="maskband")
        nc.sync.dma_start(out=maskband, in_=mb_h[:, :])
        eshift = consts.tile([65, 64], BF16, name="eshift")
        nc.sync.dma_start(out=eshift, in_=esh_h[:, :])
        wpj_sb = wts.tile([128, 4, NX], BF16, name="wpj_sb")
        nc.sync.dma_start(out=wpj_sb, in_=wpj_h[:, :, :])

        # ---- persistent activation tiles ----
        qT = [qkp.tile([128, T], BF16, name=f"qT{p}") for p in range(NPAIR)]
        kT = [qkp.tile([128, Tc + T], BF16, name=f"kT{p}") for p in range(NPAIR)]
        # v_aug column 0 is ones -> PV psum row 0 = softmax denominator
        v_sb = [vtp.tile([128, 8, 65], BF16, name=f"v{c}") for c in range(8)]
        for c in range(8):
            nc.gpsimd.memset(v_sb[c][:, :, 0:1], 1.0)
        aT = [atp_sb.tile([128, T], BF16, name=f"aT{p}") for p in range(NPAIR)]
        tmp = [atp_sb.tile([65, 2, T], BF16, name=f"tmp{p}") for p in range(NPAIR)]

        # ---- ctx projections: kcT (per pair) + vc ----
        for f in range(NPAIR):
            pt = psg.tile([128, 512], F32, tag="gen", name=f"pkc{f}")
            for kc in range(8):
                nc.tensor.matmul(
                    pt[:, 0:Tc], wkc_sb[:, kc, 128 * f:128 * f + 128],
                    ctx_sb[:, kc, :], start=(kc == 0), stop=(kc == 7),
                    skip_group_check=True)
            nc.scalar.activation(
                out=kT[f][:, 0:Tc], in_=pt[:, 0:Tc], func=AF.Identity,
                bias=bias_kc[:, f:f + 1], scale=1.0)
        pt = psg.tile([128, 512], F32, tag="gen", name="pvc")
        for kc in range(8):
            nc.tensor.matmul(
                pt[:, 0:512], ctx_sb[:, kc, :], wvc_sb[:, kc, :],
                start=(kc == 0), stop=(kc == 7), skip_group_check=True)
        nc.vector.tensor_add(
            out=v_sb[0][:, :, 1:65],
            in0=pt[:, 0:512].rearrange("p (h d) -> p h d", h=8),
            in1=bvcb.rearrange("p (h d) -> p h d", h=8))

        # ---- v projection (natural layout) ----
        for tt in range(7):
            pt = psg.tile([128, 512], F32, tag="gen", name=f"pv{tt}")
            for kc in range(8):
                nc.tensor.matmul(
                    pt[:, 0:512], x_sb[:, kc, 128 * tt:128 * tt + 128],
                    wv_sb[:, kc, :], start=(kc == 0), stop=(kc == 7),
                    skip_group_check=True)
            nc.vector.tensor_add(
                out=v_sb[1 + tt][:, :, 1:65],
                in0=pt[:, 0:512].rearrange("p (h d) -> p h d", h=8),
                in1=bvb.rearrange("p (h d) -> p h d", h=8))

        # ---- qT / kT projections, as a generator for interleaving ----
        def gen_qk(p):
            for wsb, dest, dcol, bcol in ((wq_sb, qT[p], 0, p),
                                          (wk_sb, kT[p], Tc, 4 + p)):
                for ts, te in TCH:
                    pt = psg.tile([128, 512], F32, tag="gen",
                                  name=f"pqk{p}{dcol}{ts}")
                    for kc in range(8):
                        nc.tensor.matmul(
                            pt[:, 0:te - ts], wsb[:, kc, 128 * p:128 * p + 128],
                            x_sb[:, kc, ts:te], start=(kc == 0), stop=(kc == 7),
                            skip_group_check=True)
                        yield
                    nc.scalar.activation(
                        out=dest[:, dcol + ts:dcol + te], in_=pt[:, 0:te - ts],
                        func=AF.Identity, bias=bias_qk[:, bcol:bcol + 1],
                        scale=1.0)
                    yield

        # ---- out projection per sweep, as a generator ----
        def gen_outproj(si):
            ts, te, _ = SWEEPS[si]
            for of in range(8):
                pt = psg.tile([128, 512], F32, tag="gen", name=f"po{si}{of}")
                for kc in range(NPAIR):
                    nc.tensor.matmul(
                        pt[:, 0:te - ts], wpj_sb[:, kc, 128 * of:128 * of + 128],
                        aT[kc][:, ts:te], start=(kc == 0), stop=(kc == 3),
                        skip_group_check=True)
                    yield
                ob = outp.tile([128, 512], F32, tag="ob", name=f"ob{si}{of}")
                if si == 0:  # ACT is exp-busy mid-kernel (gpsimd can't read PSUM)
                    nc.vector.tensor_copy(out=ob[:, 0:te - ts], in_=pt[:, 0:te - ts])
                else:
                    nc.scalar.copy(out=ob[:, 0:te - ts], in_=pt[:, 0:te - ts])
                yield
                nc.sync.dma_start(out=out_h[128 * of:128 * of + 128, ts:te],
                                  in_=ob[:, 0:te - ts])
                yield

        fillers = deque()

        def drain(n):
            for _ in range(n):
                if not fillers:
                    return
                gen = fillers[0]
                try:
                    next(gen)
                except StopIteration:
                    fillers.popleft()

        # ---- attention for one (pair, sweep) ----
        def attention(p, si, slots_budget):
            ts, te, last_c = SWEEPS[si]
            chunks = [c for c in range(8) if _rect(c, ts, te) is not None]
            nch = len(chunks)
            at_ps = atps.tile([65, 2, 512], F32, tag="at", name=f"at{p}{si}")
            for c in chunks:
                cs, _ = _rect(c, ts, te)
                sc = scp.tile([128, 2, 512], F32, tag="sc", name=f"sc{p}{si}{c}")
                for hi in range(2):
                    nc.tensor.matmul(
                        sc[:, hi, cs - ts:te - ts],
                        kT[p][64 * hi:64 * hi + 64, 128 * c:128 * c + 128],
                        qT[p][64 * hi:64 * hi + 64, cs:te],
                        start=True, stop=True, tile_position=(64 * hi, 0))
                pb = probsp.tile([128, 2, 512], BF16, tag="pb", name=f"pb{p}{si}{c}")
                nc.scalar.activation(
                    out=pb[:, :, cs - ts:te - ts], in_=sc[:, :, cs - ts:te - ts],
                    func=AF.Exp,
                    bias=(ebias[:, 1:2] if c == 0 else ebias[:, 0:1]),
                    scale=0.125)
                for hi in range(2):
                    for s0, e0, mc in _band_pieces(c, ts, te):
                        nc.vector.tensor_mul(
                            out=pb[:, hi, s0 - ts:e0 - ts],
                            in0=pb[:, hi, s0 - ts:e0 - ts],
                            in1=maskband[:, mc:mc + (e0 - s0)])
                # PE fillers hide the exp+mask latency before this chunk's PV
                drain(-(-slots_budget // nch))
                for hi in range(2):
                    nc.tensor.matmul(
                        at_ps[0:65, hi, cs - ts:te - ts],
                        v_sb[c][:, 2 * p + hi, :],
                        pb[:, hi, cs - ts:te - ts],
                        start=(c == 0), stop=(c == last_c),
                        skip_group_check=True)
            # evacuate aT(+denominator row 0) to SBUF as bf16
            nc.vector.tensor_copy(out=tmp[p][0:65, :, ts:te],
                                  in_=at_ps[0:65, :, 0:te - ts])

        # ---- per-(pair, sweep) normalization + aT assembly: all on-chip ----
        def gen_asm(p, si):
            ts, te, _ = SWEEPS[si]
            w = te - ts
            # broadcast denominators (psum row 0 of each head) to di[64hi..]
            di = psg.tile([128, 512], F32, tag="gen", name=f"di{p}{si}")
            for hi in range(2):
                nc.tensor.matmul(
                    di[64 * hi:64 * hi + 64, 0:w], ones_c,
                    tmp[p][0:1, hi, ts:te], start=True, stop=True,
                    tile_position=(0, 64 * hi), skip_group_check=True)
                yield
            rbc = misc.tile([128, 512], F32, tag="rbc", name=f"rbc{p}{si}")
            nc.vector.reciprocal_approx_fast(out=rbc[:, 0:w], in_=di[:, 0:w])
            yield
            # shift heads onto partitions 0-64 / 64-128 (rows 1-64 of tmp)
            aps = psg.tile([128, 512], F32, tag="gen", name=f"aps{p}{si}")
            for hi in range(2):
                nc.tensor.matmul(
                    aps[64 * hi:64 * hi + 64, 0:w], eshift,
                    tmp[p][0:65, hi, ts:te], start=True, stop=True,
                    tile_position=(0, 64 * hi), skip_group_check=True)
                yield
            nc.vector.tensor_mul(out=aT[p][:, ts:te], in0=aps[:, 0:w],
                                 in1=rbc[:, 0:w])
            yield

        # ---- phase program ----
        for _ in gen_qk(0):  # emit qk(0) directly
            pass
        for si in range(2):
            for p in range(NPAIR):
                if si == 0:
                    if p + 1 < NPAIR:
                        fillers.append(gen_qk(p + 1))
                    budget = 44 if p + 1 < NPAIR else 8
                else:
                    budget = 16
                attention(p, si, slots_budget=budget)
                fillers.appendleft(gen_asm(p, si))
            drain(10**6)
            if si == 0:
                fillers.append(gen_outproj(0))
        drain(10**6)
        for _ in gen_outproj(1):  # tail: tokens 512:896
            pass

    if not nc.is_finalized():
        nc.finalize()
    return nc


_NC_CACHE = {}


def _get_nc():
    if "nc" not in _NC_CACHE:
        _NC_CACHE["nc"] = build_nc()
    return _NC_CACHE["nc"]


def _pack128(v):
    """[128*n] -> [128, n] with [p, f] = v[128*f + p]."""
    n = v.shape[0] // 128
    return np.ascontiguousarray(v.reshape(n, 128).T)


def _perm(w2d, dtype):
    """[128*kc, f] -> [128, kc, f] contiguous (SBUF layout built on host)."""
    kc = w2d.shape[0] // 128
    return np.ascontiguousarray(
        w2d.reshape(kc, 128, w2d.shape[1]).transpose(1, 0, 2).astype(dtype))


def make_in_maps(inputs):
    bf16 = ml_dtypes.bfloat16
    x = np.asarray(inputs["x"], np.float32)
    ctx_seq = np.asarray(inputs["context_seq"], np.float32)
    w_ref = np.asarray(inputs["w_ref"], np.float32)
    b_ref = np.asarray(inputs["b_ref"], np.float32)
    w_attn = np.asarray(inputs["w_attn"], np.float32)
    b_attn = np.asarray(inputs["b_attn"], np.float32)
    w_proj = np.asarray(inputs["w_proj"], np.float32)

    # mask band constant: cols 0-127 causal (1 where q>=p), cols 128-255
    # anti-diagonal (0 where q==p else 1)
    qq = np.arange(128)[None, :]
    pp = np.arange(128)[:, None]
    mband = np.concatenate([(qq >= pp), (qq != pp)], axis=1).astype(bf16)
    mband = np.ascontiguousarray(mband)
    eshift = np.ascontiguousarray(np.eye(65, 64, k=-1).astype(bf16))

    in_maps = []
    for b in range(4):
        xr = _perm(x[b].T, bf16)
        ctxr = _perm(ctx_seq[b].T, bf16)
        for g in range(2):
            sl = slice(512 * g, 512 * g + 512)
            in_maps.append(dict(
                xr=xr,
                ctxr=ctxr,
                w_q=_perm(w_attn[:, 0 * NX:1 * NX][:, sl], bf16),
                w_k=_perm(w_attn[:, 1 * NX:2 * NX][:, sl], bf16),
                w_v=_perm(w_attn[:, 2 * NX:3 * NX][:, sl], bf16),
                w_kc=_perm(w_ref[:, 0 * NX:1 * NX][:, sl], bf16),
                w_vc=_perm(w_ref[:, 1 * NX:2 * NX][:, sl], bf16),
                w_pj=_perm(w_proj[sl, :], bf16),
                b_qk=_pack128(np.concatenate([b_attn[0 * NX:1 * NX][sl],
                                              b_attn[1 * NX:2 * NX][sl]])),
                b_kc=_pack128(b_ref[0 * NX:1 * NX][sl]),
                b_v=np.ascontiguousarray(b_attn[2 * NX:3 * NX][sl].reshape(1, 512)),
                b_vc=np.ascontiguousarray(b_ref[1 * NX:2 * NX][sl].reshape(1, 512)),
                mband=mband,
                eshift=eshift,
            ))
    return in_maps


def kernel(**inputs):
    b_proj = np.asarray(inputs["b_proj"], np.float32)
    in_maps = make_in_maps(inputs)
    nc = _get_nc()
    res = run_bass_kernel_spmd(nc, in_maps, core_ids=list(range(8)),
                               trace=os.environ.get("COCON_TRACE", "") == "1")
    outs = res.results
    out = np.empty((4, T, NX), np.float32)
    for b in range(4):
        acc = outs[2 * b]["outT"] + outs[2 * b + 1]["outT"]  # [1024, 896]
        out[b] = acc.T + b_proj[None, :]
    if res.exec_time_ns is not None:
        kernel.last_exec_time_ns = res.exec_time_ns
    return out


kernel.last_exec_time_ns = None
